# revision 8
# baseline (speedup 1.0000x reference)
"""GCN-Tox21 GNN message-passing kernel for 8 Trainium2 NeuronCores (v2).

Strategy (graph/edge parallelism, edges sorted by destination):
  - batch is sorted, so core k owns the nodes of graphs [k*64,(k+1)*64) --
    a contiguous node range, padded to a multiple of 128 (NPC rows in the
    replicated h table). Edges are sorted by dst and assigned to the core
    owning their dst; within a core they are grouped into 128-node windows
    and padded to 128-edge subtiles (subtile count per window = max over
    cores, so the SPMD instruction stream is identical on every core).
  - Node features h live replicated in DRAM in fp8(e4m3). Per-edge h[src]
    rows are fetched with dma_gather(transpose=True); its 16-bit-granular
    transpose lands feature pairs interleaved -- exactly the lhsT layout
    DoubleRowSwInterleave expects. Gather indices are host-reversed per
    128-subtile to cancel SwInterleave's column reversal.
  - Per-edge pre-activation y[e, mid] accumulates in PSUM edge-major:
      src: one fp8 SwInterleave matmul (K=256 in one instruction)
      dst: one-hot expand, resident fp8 S^T tile x per-window Q (bf16)
      e:   K=17 matmul vs resident W1e (+b1 via ones row of e^T)
  - z = relu(y); segment-sum to nodes via resident fp8 one-hot S tiles.
    W2 is linear so it is applied AFTER the segment mean, once per
    128-node window (scale by 1/cnt, transpose, 2-4 matmuls, +c_row)
    instead of per edge. BN folds into W2 and c_row.
  - h_next: fp8 to DRAM (AllGather across cores rebuilds the replicated
    table) + bf16 transposed into SBUF for the next layer's Q.
  - Final layer: no AllGather. Each core's stripe contains exactly its own
    64 graphs, so mean-pool is one 64-wide one-hot matmul per window into
    a PSUM accumulator, then FC; host applies the exact sigmoid.
"""

import numpy as np
import ml_dtypes

import concourse.bacc as bacc
import concourse.tile as tile
from concourse import mybir, bass_utils
from concourse.masks import make_identity

BF16 = mybir.dt.bfloat16
F32 = mybir.dt.float32
FP8 = mybir.dt.float8e4
RELU = mybir.ActivationFunctionType.Relu
COPY = mybir.ActivationFunctionType.Copy
SWIL = mybir.MatmulPerfMode.DoubleRowSwInterleave

N_CORES = 8
BN_EPS = 1e-5
G_REAL = 512
F_NODE, F_EDGE, H, EH = 32, 8, 256, 16
OUT_DIMS = (256, 256, 128)
EG = 512  # edges per gather batch

NP_FP8 = ml_dtypes.float8_e4m3


def _bf(a):
    return np.ascontiguousarray(np.asarray(a).astype(ml_dtypes.bfloat16))


def _f8(a):
    return np.ascontiguousarray(np.asarray(a).astype(NP_FP8))


def _f32(a):
    return np.ascontiguousarray(np.asarray(a).astype(np.float32))


def _wrap_idx(idx):
    """int16 index layout for dma_gather: index i at [i % 16, i // 16],
    replicated across the 8 partition groups."""
    assert len(idx) % 16 == 0
    w = idx.astype(np.int16).reshape(-1, 16).T
    return np.ascontiguousarray(np.tile(w, (8, 1)))


class Plan:
    """Host-side preprocessing: sharding layout + per-core input tensors."""

    def __init__(self, inputs, G):
        x = np.asarray(inputs["x"]).astype(np.float32)
        N = x.shape[0]
        self.N, self.G = N, G
        assert G % N_CORES == 0
        self.GPC = G // N_CORES

        edge_index = np.asarray(inputs["edge_index"]).astype(np.int64)
        src, dst = edge_index[0].astype(np.int32), edge_index[1].astype(np.int32)
        batch = np.asarray(inputs["batch"]).astype(np.int32)
        edge_attr = np.asarray(inputs["edge_attr"]).astype(np.float32)

        # graph-aligned node stripes
        lo = [int(np.searchsorted(batch, k * self.GPC, "left"))
              for k in range(N_CORES)] + [N]
        L = [lo[k + 1] - lo[k] for k in range(N_CORES)]
        self.W = max(-(-Lk // 128) for Lk in L)
        self.NPC = self.W * 128
        self.N_tab = N_CORES * self.NPC
        self.lo = lo

        core_of = np.searchsorted(np.asarray(lo[1:]), np.arange(N), "right")
        pos_of = (core_of * self.NPC + np.arange(N)
                  - np.asarray(lo)[core_of]).astype(np.int32)

        order = np.argsort(dst, kind="stable")
        s_dst, s_src = dst[order], src[order]
        s_ea = edge_attr[order]

        # window boundaries in sorted-dst space (node-id bounds per window)
        win_lo = np.zeros(N_CORES * self.W, np.int64)
        win_hi = np.zeros(N_CORES * self.W, np.int64)
        for k in range(N_CORES):
            for w in range(self.W):
                a = min(lo[k] + w * 128, lo[k + 1])
                b = min(lo[k] + (w + 1) * 128, lo[k + 1])
                gw = k * self.W + w
                win_lo[gw] = np.searchsorted(s_dst, a, "left")
                win_hi[gw] = np.searchsorted(s_dst, b, "left")
        cnt_w = (win_hi - win_lo).reshape(N_CORES, self.W)
        T_w = np.maximum(1, -(-cnt_w.max(axis=0) // 128))
        while T_w.sum() % (EG // 128) != 0:
            T_w[-1] += 1
        self.T_w = [int(t) for t in T_w]
        self.T_tot = int(T_w.sum())
        self.ET = self.T_tot * 128

        cnt = np.bincount(dst, minlength=N).astype(np.float32)
        invc_full = 1.0 / np.maximum(cnt, 1.0)
        gcnt = np.bincount(batch, minlength=G).astype(np.float32)
        ginv_full = 1.0 / np.maximum(gcnt, 1.0)

        self.per_core = []
        for k in range(N_CORES):
            d = {}
            gi_src = np.zeros(self.ET, np.int32)
            ea_pad = np.zeros((self.ET, F_EDGE), np.float32)
            S = np.zeros((128, self.ET), np.float32)
            invc = np.ones((self.W, 128), np.float32)
            S2 = np.zeros((128, self.W * self.GPC), np.float32)
            pos = 0
            for w in range(self.W):
                base = lo[k] + w * 128
                nreal = min(128, max(0, lo[k + 1] - base))
                if nreal > 0:
                    invc[w, :nreal] = invc_full[base:base + nreal]
                    gg = batch[base:base + nreal] - k * self.GPC
                    S2[np.arange(nreal), w * self.GPC + gg] = 1.0
                a, b = win_lo[k * self.W + w], win_hi[k * self.W + w]
                n = b - a
                sl = slice(pos, pos + n)
                gi_src[sl] = pos_of[s_src[a:b]]
                ea_pad[sl] = s_ea[a:b]
                locd = (s_dst[a:b] - base).astype(np.int64)
                e_ids = np.arange(pos, pos + n)
                S[e_ids % 128, (e_ids // 128) * 128 + locd] = 1.0
                pos += self.T_w[w] * 128
            assert pos == self.ET

            # reverse gather order within each 128-subtile: SwInterleave
            # reads columns last-first, so psum row m = natural edge m
            gi_rev = gi_src.reshape(-1, 128)[:, ::-1].reshape(-1)
            d["gidx_src"] = _wrap_idx(gi_rev)
            d["S"] = _f8(S)
            ST = np.ascontiguousarray(
                S.reshape(128, self.T_tot, 128).transpose(2, 1, 0)
                .reshape(128, self.ET))
            d["S_T"] = _f8(ST)
            eaT = np.concatenate([ea_pad.T, np.ones((1, self.ET), np.float32)], 0)
            d["eaT"] = _bf(eaT)
            d["invc"] = _f32(invc.T)
            d["S2"] = _f8(S2)
            d["ginv"] = _f32(ginv_full[k * self.GPC:(k + 1) * self.GPC]
                             .reshape(self.GPC, 1))
            self.per_core.append(d)

        sh = {}
        # x in table layout [33, N_tab] (pad cols zero)
        xT_tab = np.zeros((F_NODE + 1, self.N_tab), np.float32)
        xT_tab[F_NODE, :] = 1.0
        xT_tab[:F_NODE, pos_of] = x.T
        sh["xT"] = _bf(xT_tab)
        for k in range(N_CORES):
            self.per_core[k]["xT_own"] = _bf(
                xT_tab[:, k * self.NPC:(k + 1) * self.NPC])
        ne_w, ne_b = _f32(inputs["ne_w"]), _f32(inputs["ne_b"])
        sh["ne_wT"] = _bf(np.concatenate([ne_w.T, ne_b[None, :]], 0))
        ee_w, ee_b = _f32(inputs["ee_w"]), _f32(inputs["ee_b"])
        sh["ee_wT"] = _bf(np.concatenate([ee_w.T, ee_b[None, :]], 0))

        in_dim = H
        self.layer_dims = []
        for i, out_dim in enumerate(OUT_DIMS):
            w1 = _f32(inputs[f"c{i}_w1"]); b1 = _f32(inputs[f"c{i}_b1"])
            w2 = _f32(inputs[f"c{i}_w2"]); b2 = _f32(inputs[f"c{i}_b2"])
            g = _f32(inputs[f"bn{i}_g"]); bb = _f32(inputs[f"bn{i}_b"])
            rm = _f32(inputs[f"bn{i}_m"]); rv = _f32(inputs[f"bn{i}_v"])
            A = g / np.sqrt(rv + BN_EPS)
            F_mid = 2 * out_dim
            w1T = w1.T  # [2*in + EH, F_mid]; K-order: [h_dst, h_src, e]
            sh[f"w1dT_{i}"] = _bf(w1T[0:in_dim, :])
            # src rows in SwInterleave rhs layout [kpair, 2, F_mid]
            sh[f"w1il_{i}"] = _f8(w1T[in_dim:2 * in_dim, :]
                                  .reshape(in_dim // 2, 2, F_mid))
            sh[f"w1eT_{i}"] = _bf(
                np.concatenate([w1T[2 * in_dim:, :], b1[None, :]], 0))
            sh[f"w2T_{i}"] = _bf((w2 * A[:, None]).T)
            sh[f"crow_{i}"] = _bf((b2 * A + bb - rm * A)[None, :])
            self.layer_dims.append((in_dim, F_mid, out_dim))
            in_dim = out_dim

        fc_w, fc_b = _f32(inputs["fc_w"]), _f32(inputs["fc_b"])
        self.F_FC = fc_w.shape[0]
        sh["fc_wT"] = _bf(fc_w.T)
        sh["fcb_bc"] = _f32(np.tile(fc_b[None, :], (self.GPC, 1)))
        self.shared = sh

    def in_maps(self):
        return [{**self.shared, **self.per_core[k]} for k in range(N_CORES)]


def build_program(plan: Plan, n_cores=N_CORES, debug_no_collective=False,
                  debug_stage=9, repeats=1, skip_gather=False,
                  skip_compute=False, seq_dma=False):
    nc = bacc.Bacc("TRN2", target_bir_lowering=False, debug=False,
                   num_devices=n_cores)

    ET, T_w, W, NPC, GPC = plan.ET, plan.T_w, plan.W, plan.NPC, plan.GPC
    N_tab, F_FC = plan.N_tab, plan.F_FC
    n_batches = ET // EG

    sub_window, sub_first, sub_last = [], [], []
    for w in range(W):
        for t in range(T_w[w]):
            sub_window.append(w)
            sub_first.append(t == 0)
            sub_last.append(t == T_w[w] - 1)

    sample = plan.in_maps()[0]
    t_in = {name: nc.dram_tensor(name, list(arr.shape),
                                 mybir.dt.from_np(arr.dtype),
                                 kind="ExternalInput")
            for name, arr in sample.items()}
    out_part = nc.dram_tensor("out_part", [GPC, F_FC], F32,
                              kind="ExternalOutput")

    with tile.TileContext(nc) as tc:
        with (
            tc.tile_pool(name="const", bufs=1) as cpool,
            tc.tile_pool(name="sbuf", bufs=2) as spool,
            tc.tile_pool(name="gath", bufs=6) as gpool,
            tc.tile_pool(name="zpool", bufs=6) as zpool,
            tc.tile_pool(name="psum", bufs=2, space="PSUM") as ppool,
            tc.tile_pool(name="dram", bufs=1, space="DRAM") as dpool,
        ):
            def _body():
                # ---------- resident constants ----------
                def load_const(name, tag=None):
                    arr = sample[name]
                    t = cpool.tile(list(arr.shape), mybir.dt.from_np(arr.dtype),
                                   tag=tag or name)
                    nc.sync.dma_start(out=t[:], in_=t_in[name][:])
                    return t

                S_t = load_const("S")
                ST_t = load_const("S_T")
                gidx_src = load_const("gidx_src")
                invc_t = load_const("invc")
                S2_t = load_const("S2")
                ginv_t = load_const("ginv")
                ne_wT_t = load_const("ne_wT")
                ee_wT_t = load_const("ee_wT")
                fc_wT_t = load_const("fc_wT")
                fcb_t = load_const("fcb_bc")
                ident = cpool.tile([128, 128], BF16, tag="ident")
                make_identity(nc, ident[:])
                ones1 = cpool.tile([1, 128], BF16, tag="ones1")
                nc.vector.memset(ones1[:], 1.0)

                w1dT_t, w1il_t, w1eT_t, w2T_t, crow_t = [], [], [], [], []
                for i, (F_in, F_mid, F_out) in enumerate(plan.layer_dims):
                    chunks = []
                    for kc in range(F_in // 128):
                        t = cpool.tile([128, F_mid], BF16, tag=f"w1dT_{i}_{kc}")
                        nc.sync.dma_start(
                            out=t[:],
                            in_=t_in[f"w1dT_{i}"][kc * 128:(kc + 1) * 128, :])
                        chunks.append(t)
                    w1dT_t.append(chunks)
                    w1il_t.append(load_const(f"w1il_{i}"))
                    w1eT_t.append(load_const(f"w1eT_{i}"))
                    wc = []
                    for km in range(F_mid // 128):
                        t = cpool.tile([128, F_out], BF16, tag=f"w2T_{i}_{km}")
                        nc.sync.dma_start(
                            out=t[:],
                            in_=t_in[f"w2T_{i}"][km * 128:(km + 1) * 128, :])
                        wc.append(t)
                    w2T_t.append(wc)
                    crow_t.append(load_const(f"crow_{i}"))

                hT_t = [cpool.tile([128, W, d[0] // 128, 128], BF16,
                                   tag=f"hT{i}", name=f"hT{i}")
                        for i, d in enumerate(plan.layer_dims)]
                qbuf = cpool.tile([128, W, 512], BF16, tag="qbuf")

                # ---------- DRAM buffers ----------
                h_full = [dpool.tile([N_tab, 256], FP8, tag=f"h{i}",
                                     name=f"h_full{i}") for i in range(3)]
                h_own = [dpool.tile([NPC, 256], FP8, tag=f"hown{i}",
                                    name=f"h_own{i}") for i in range(2)]
                eT_dram = dpool.tile([EH + 1, ET], BF16, tag="eT")

                # ---------- stage A: h0 = relu(x @ ne_w.T + ne_b) ----------
                for chunk in range(N_tab // 128):
                    n0 = chunk * 128
                    xt = spool.tile([F_NODE + 1, 128], BF16, tag="xT", bufs=3)
                    nc.sync.dma_start(out=xt[:], in_=t_in["xT"][:, n0:n0 + 128])
                    ps = ppool.tile([128, H], F32, tag="m1")
                    nc.tensor.matmul(out=ps[:], lhsT=xt[:], rhs=ne_wT_t[:],
                                     start=True, stop=True)
                    h0sb = spool.tile([128, H], FP8, tag="h0sb")
                    nc.scalar.activation(out=h0sb[:], in_=ps[:], func=RELU)
                    nc.sync.dma_start(out=h_full[0][n0:n0 + 128, :], in_=h0sb[:])
                # own-stripe h0 again, transposed into hT_t[0]
                for w in range(W):
                    xo = spool.tile([F_NODE + 1, 128], BF16, tag="xT", bufs=3)
                    nc.sync.dma_start(out=xo[:],
                                      in_=t_in["xT_own"][:, w * 128:(w + 1) * 128])
                    ps = ppool.tile([128, H], F32, tag="m1")
                    nc.tensor.matmul(out=ps[:], lhsT=xo[:], rhs=ne_wT_t[:],
                                     start=True, stop=True)
                    h0o = spool.tile([128, H], BF16, tag="h0o")
                    nc.scalar.activation(out=h0o[:], in_=ps[:], func=RELU)
                    for kc in range(H // 128):
                        tp = ppool.tile([128, 128], BF16, tag="tp", bufs=1)
                        nc.tensor.transpose(out=tp[:],
                                            in_=h0o[:, kc * 128:(kc + 1) * 128],
                                            identity=ident[:])
                        nc.vector.tensor_copy(out=hT_t[0][:, w, kc, :], in_=tp[:])

                # ---------- stage A2: e^T (+ones row) -> DRAM ----------
                if debug_stage < 2:
                    return
                ones_row = cpool.tile([1, 512], BF16, tag="ones512")
                nc.vector.memset(ones_row[:], 1.0)
                for g0 in range(ET // 512):
                    ea_t = spool.tile([F_EDGE + 1, 512], BF16, tag="eaT")
                    nc.sync.dma_start(out=ea_t[:],
                                      in_=t_in["eaT"][:, g0 * 512:(g0 + 1) * 512])
                    ps = ppool.tile([EH, 512], F32, tag="m1")
                    nc.tensor.matmul(out=ps[:], lhsT=ee_wT_t[:], rhs=ea_t[:],
                                     start=True, stop=True)
                    et_sb = spool.tile([EH, 512], BF16, tag="etsb")
                    nc.scalar.activation(out=et_sb[:], in_=ps[:], func=RELU)
                    nc.sync.dma_start(out=eT_dram[0:EH, g0 * 512:(g0 + 1) * 512],
                                      in_=et_sb[:])
                    nc.sync.dma_start(
                        out=eT_dram[EH:EH + 1, g0 * 512:(g0 + 1) * 512],
                        in_=ones_row[:])

                # ---------- conv layers ----------
                if debug_stage < 3:
                    return
                gs_const = None
                if skip_gather:
                    gs_const = cpool.tile([128, 2, EG], FP8, tag="gs_const")
                    nc.vector.memset(gs_const[:], 0.25)

                pool_ps = None
                for li, (F_in, F_mid, F_out) in enumerate(plan.layer_dims):
                    h_in = h_full[li]
                    MCw = F_mid // 128
                    last_layer = (li == 2)

                    # per-window Q (overlaps the previous AllGather)
                    for w in range(W):
                        qtp = ppool.tile([128, F_mid], F32, tag="qt", bufs=1)
                        for kc in range(F_in // 128):
                            nc.tensor.matmul(out=qtp[:],
                                             lhsT=hT_t[li][:, w, kc, :],
                                             rhs=w1dT_t[li][kc][:],
                                             start=(kc == 0),
                                             stop=(kc == F_in // 128 - 1))
                        nc.vector.tensor_copy(out=qbuf[:, w, 0:F_mid],
                                              in_=qtp[:])

                    node_ps = None
                    for b in range(n_batches):
                        e0 = b * EG
                        if skip_gather:
                            gs = gs_const
                        else:
                            gs = gpool.tile([128, 2, EG], FP8, tag="gs")
                            if seq_dma:
                                src0 = (b * 256) % (N_tab - 512)
                                for q in range(2):
                                    nc.sync.dma_start(
                                        out=gs[:, q, :],
                                        in_=h_in[src0 + q * 256:
                                                 src0 + q * 256 + 256, :]
                                        .rearrange("a b -> (a b)")
                                        .rearrange("(p f) -> p f", p=128))
                            else:
                                nc.gpsimd.dma_gather(
                                    gs[:], h_in[:, :],
                                    gidx_src[:, e0 // 16:(e0 + EG) // 16],
                                    EG, EG, 256, transpose=True)
                        if skip_compute:
                            continue
                        gs_flat = gs[:].rearrange("p c i -> p (c i)")
                        et_t = spool.tile([EH + 1, 512], BF16, tag="et_in",
                                          bufs=4)
                        nc.sync.dma_start(out=et_t[:],
                                          in_=eT_dram[:, b * 512:(b + 1) * 512])
                        for s in range(4):
                            t_glob = b * 4 + s
                            w = sub_window[t_glob]
                            ps = ppool.tile([128, F_mid], F32, tag="m1")
                            nc.tensor.matmul(
                                out=ps[:],
                                lhsT=gs_flat[:, s * 256:(s + 1) * 256],
                                rhs=w1il_t[li][:], start=True, stop=False,
                                perf_mode=SWIL, skip_group_check=True)
                            nc.tensor.matmul(
                                out=ps[:],
                                lhsT=ST_t[:, t_glob * 128:(t_glob + 1) * 128],
                                rhs=qbuf[:, w, 0:F_mid], start=False,
                                stop=False, skip_group_check=True)
                            nc.tensor.matmul(
                                out=ps[:], lhsT=et_t[:, s * 128:(s + 1) * 128],
                                rhs=w1eT_t[li][:], start=False, stop=True,
                                skip_group_check=True)
                            z = zpool.tile([128, F_mid], BF16, tag="z")
                            nc.scalar.activation(out=z[:], in_=ps[:], func=RELU)
                            if sub_first[t_glob]:
                                node_ps = ppool.tile([128, F_mid], F32,
                                                     tag="node")
                            nc.tensor.matmul(
                                out=node_ps[:],
                                lhsT=S_t[:, t_glob * 128:(t_glob + 1) * 128],
                                rhs=z[:], start=sub_first[t_glob],
                                stop=sub_last[t_glob], skip_group_check=True)
                            if not sub_last[t_glob]:
                                continue
                            # ---- window w done: mean + W2 + BN + relu ----
                            zn = spool.tile([128, F_mid], BF16, tag="zn")
                            nc.scalar.activation(out=zn[:], in_=node_ps[:],
                                                 func=COPY,
                                                 scale=invc_t[:, w:w + 1])
                            znT = spool.tile([128, MCw, 128], BF16, tag="znT",
                                             bufs=3)
                            for kc in range(MCw):
                                tp = ppool.tile([128, 128], BF16, tag="tp", bufs=1)
                                nc.tensor.transpose(
                                    out=tp[:],
                                    in_=zn[:, kc * 128:(kc + 1) * 128],
                                    identity=ident[:])
                                nc.vector.tensor_copy(out=znT[:, kc, :],
                                                      in_=tp[:])
                            ps2 = ppool.tile([128, F_out], F32, tag="m2", bufs=1)
                            nc.tensor.matmul(out=ps2[:], lhsT=ones1[:],
                                             rhs=crow_t[li][:], start=True,
                                             stop=False, skip_group_check=True)
                            for kc in range(MCw):
                                nc.tensor.matmul(out=ps2[:], lhsT=znT[:, kc, :],
                                                 rhs=w2T_t[li][kc][:],
                                                 start=False,
                                                 stop=(kc == MCw - 1),
                                                 skip_group_check=True)
                            hsb = spool.tile([128, F_out], BF16, tag="hsb",
                                             bufs=3)
                            nc.scalar.activation(out=hsb[:], in_=ps2[:],
                                                 func=RELU)
                            if not last_layer:
                                h8 = spool.tile([128, F_out], FP8, tag="h8",
                                                bufs=3)
                                nc.vector.tensor_copy(out=h8[:], in_=hsb[:])
                                nc.sync.dma_start(
                                    out=h_own[li][w * 128:(w + 1) * 128, :],
                                    in_=h8[:])
                                for kc in range(F_out // 128):
                                    tp = ppool.tile([128, 128], BF16, tag="tp", bufs=1)
                                    nc.tensor.transpose(
                                        out=tp[:],
                                        in_=hsb[:, kc * 128:(kc + 1) * 128],
                                        identity=ident[:])
                                    nc.vector.tensor_copy(
                                        out=hT_t[li + 1][:, w, kc, :],
                                        in_=tp[:])
                            else:
                                if pool_ps is None:
                                    pool_ps = ppool.tile([GPC, 128], F32,
                                                         tag="pool", bufs=1)
                                nc.tensor.matmul(
                                    out=pool_ps[:],
                                    lhsT=S2_t[:, w * GPC:(w + 1) * GPC],
                                    rhs=hsb[:], start=(w == 0),
                                    stop=(w == W - 1), skip_group_check=True)
                    if skip_compute:
                        continue
                    if not last_layer:
                        if debug_no_collective:
                            cp = spool.tile([128, 256], FP8, tag="dbgcp")
                            nc.sync.dma_start(out=cp[:], in_=h_own[li][0:128, :])
                            nc.sync.dma_start(out=h_full[li + 1][0:128, :],
                                              in_=cp[:])
                        else:
                            nc.gpsimd.collective_compute(
                                "AllGather", mybir.AluOpType.bypass,
                                ins=[h_own[li].opt()],
                                outs=[h_full[li + 1].opt()],
                                replica_groups=[list(range(n_cores))])

                # ---------- tail: graph mean + FC ----------
                if debug_stage < 5 or skip_compute:
                    return
                F_last = plan.layer_dims[-1][2]
                pooled_sb = spool.tile([GPC, F_last], BF16, tag="pooled")
                nc.scalar.activation(out=pooled_sb[:], in_=pool_ps[:],
                                     func=COPY, scale=ginv_t[:])
                ptr_ps = ppool.tile([F_last, GPC], BF16, tag="qt", bufs=1)
                nc.tensor.transpose(out=ptr_ps[:], in_=pooled_sb[:],
                                    identity=ident[0:GPC, 0:GPC])
                ptr_sb = spool.tile([F_last, GPC], BF16, tag="ptrsb")
                nc.vector.tensor_copy(out=ptr_sb[:], in_=ptr_ps[:])
                fc_ps = ppool.tile([GPC, F_FC], F32, tag="m1")
                nc.tensor.matmul(out=fc_ps[:], lhsT=ptr_sb[:], rhs=fc_wT_t[:],
                                 start=True, stop=True)
                logit = spool.tile([GPC, F_FC], F32, tag="logit")
                nc.vector.tensor_tensor(out=logit[:], in0=fc_ps[:],
                                        in1=fcb_t[:], op=mybir.AluOpType.add)
                nc.sync.dma_start(out=out_part[:], in_=logit[:])

            for _r in range(repeats):
                _body()

    nc.compile()
    return nc


_CACHE = {}


def run(inputs, G=G_REAL):
    plan = Plan(inputs, G)
    key = (plan.N, plan.G, plan.W, tuple(plan.T_w))
    if key not in _CACHE:
        _CACHE[key] = build_program(plan)
    nc = _CACHE[key]
    res = bass_utils.run_bass_kernel_spmd(nc, plan.in_maps(),
                                          core_ids=list(range(N_CORES)))
    logits = np.concatenate([res.results[k]["out_part"]
                             for k in range(N_CORES)], 0)
    out = 1.0 / (1.0 + np.exp(-logits.astype(np.float64)))
    return np.ascontiguousarray(out.astype(np.float32))


def kernel(**inputs) -> np.ndarray:
    return run(inputs, G=G_REAL)
